# revision 41
# baseline (speedup 1.0000x reference)
"""Trainium2 Bass kernel for nn_AttModel (masked attention GNN message passing).

Contract: kernel(**inputs) takes the FULL unsharded inputs (x [8,2048,128],
mask [8,2048,2048], Wv/Wk/Wq [128,128], bv/bk/bq [128]) and returns the full
output [8, 2048, 128] float32.

Strategy: data-parallel over batch B=8 across the 8 NeuronCores; the small
weight matrices are replicated. The device kernel runs a fully transposed
dataflow (scores computed as S^T per j-stripe) so no [N,N] transpose is ever
done on device; the host pre-transposes x, mask and the weights (pure layout
marshaling) and post-normalizes/transposes the returned outT/rowsum.

Per core (batch element b), bf16 data path, f32 PSUM accumulation:
  qT/kT = relu(W x^T + b) as [h, n] bf16
  v     = relu(x W^T + b) computed directly in natural [j, h] orientation
          (bias via a K=1 ones-row matmul into PSUM; relu on DVE)
  for i-chunk (1024) and j-stripe (128):
    sT = kT_j^T @ qT_chunk      (PE, bf16, PSUM f32)
    eT = exp(sT)                (ACT, PSUM -> SBUF bf16, two stripes/tile)
    pT = eT * maskT             (DVE, one FD=2048 multiply per stripe pair)
    outT_chunk   += v_j^T @ pT  (PE, accumulated in PSUM over stripes)
    p01 = pT_even + pT_odd (DVE); rowsum += 1^T @ p01 (PE, per pair)
  Host: out_b = (outT / rowsum)^T

DMA-dispatch economy (the sync sequencer pays ~0.7us per dma_start):
inputs are packed into 3 loads (wpack, bpack, xT) + 16 full-row mask tiles
reused by both i-chunks; output stores ride the idle GPSIMD (SWDGE) ring.
"""

from contextlib import ExitStack

import numpy as np
import ml_dtypes

import concourse.bass as bass
import concourse.bacc as bacc
import concourse.tile as tile
from concourse import mybir
from concourse import bass_utils

B = 8
P = 128
N = 2048
HID = 128
DIN = 128
NJ = N // P      # 16 j-stripes
ICH = 1024       # i-chunk width
NCH = N // ICH   # 2 i-chunks
NT = NCH * NJ    # 32 global stripes

f32 = mybir.dt.float32
bf = mybir.dt.bfloat16
AF = mybir.ActivationFunctionType
ALU = mybir.AluOpType

_NC_CACHE = {}


def _attention_tile_kernel(ctx, tc, outT, rowsum, xT, maskT, wpack, bpack):
    nc = tc.nc

    consts = ctx.enter_context(tc.tile_pool(name="consts", bufs=1))
    big = ctx.enter_context(tc.tile_pool(name="big", bufs=1))
    ps = ctx.enter_context(tc.tile_pool(name="ps", bufs=1, space="PSUM"))
    e_pool = ctx.enter_context(tc.tile_pool(name="ep", bufs=4))
    p_pool = ctx.enter_context(tc.tile_pool(name="pp", bufs=5))
    pp_pool = ctx.enter_context(tc.tile_pool(name="pairp", bufs=3))
    drain_pool = ctx.enter_context(tc.tile_pool(name="drainp", bufs=2))

    # pre-warm the exp table set while DMAs are in flight
    warm_in = consts.tile([P, 1], f32)
    nc.vector.memset(warm_in, 0.0)
    warm_out = consts.tile([P, 1], bf)
    nc.scalar.activation(out=warm_out, in_=warm_in, func=AF.Exp)

    ones_col = consts.tile([P, 1], bf)
    nc.vector.memset(ones_col, 1.0)
    ones_row = consts.tile([1, P], bf)
    nc.vector.memset(ones_row, 1.0)

    # packed inputs, dispatch-ordered by criticality on the sync ring
    wp = consts.tile([P, 896], bf)
    nc.sync.dma_start(out=wp, in_=wpack)
    wts = {"q": wp[:, 0:P], "k": wp[:, P:2 * P], "v": wp[:, 2 * P:3 * P]}
    bvR = wp[0:1, 384:896]

    xT_sb = big.tile([P, N], bf)
    nc.sync.dma_start(out=xT_sb[:, 0:ICH], in_=xT[:, 0:ICH])
    xc = [xT_sb[:, c * 512:(c + 1) * 512] for c in range(4)]

    bp = consts.tile([P, 2], f32)
    nc.gpsimd.dma_start(out=bp, in_=bpack)
    biases = {"q": bp[:, 0:1], "k": bp[:, 1:2]}

    nc.sync.dma_start(out=xT_sb[:, ICH:N], in_=xT[:, ICH:N])

    # all mask rows prefetched once, reused by both i-chunks
    mask_all = big.tile([P, NJ, N], bf)
    for j in range(NJ):
        nc.sync.dma_start(out=mask_all[:, j], in_=maskT[j * P:(j + 1) * P, :])

    qT = [big.tile([P, ICH], bf, name=f"qT{c}") for c in range(NCH)]
    kT = big.tile([P, N], bf)
    vb = [big.tile([P, 512], bf, name=f"vb{c}") for c in range(4)]

    def vN(j):
        return vb[j // 4][:, (j % 4) * P:(j % 4 + 1) * P]

    def proj(nm, c):
        """Project chunk c (columns 512c:512c+512) of q/k/v; relu into SBUF."""
        pt = ps.tile([P, 512], f32, tag="s", bufs=2, name=f"proj_{nm}{c}")
        if nm == "v":
            # bias broadcast (K=1 ones matmul) then x @ WvT per stripe on top
            nc.tensor.matmul(pt, lhsT=ones_row, rhs=bvR,
                             start=True, stop=False, skip_group_check=True)
            for jj in range(4):
                nc.tensor.matmul(pt[:, jj * P:(jj + 1) * P],
                                 lhsT=xc[c][:, jj * P:(jj + 1) * P],
                                 rhs=wts["v"], start=False, stop=True,
                                 skip_group_check=True)
            nc.vector.tensor_scalar(vb[c], pt, 0.0, None, op0=ALU.max)
            return
        nc.tensor.matmul(pt, lhsT=wts[nm], rhs=xc[c], start=True, stop=True)
        if nm == "k":
            if c == 0:
                nc.scalar.activation(out=kT[:, 0:512], in_=pt,
                                     func=AF.Relu, bias=biases["k"], scale=1.0)
            else:
                nc.vector.tensor_scalar(kT[:, c * 512:(c + 1) * 512], pt,
                                        biases["k"], 0.0,
                                        op0=ALU.add, op1=ALU.max)
        else:
            dest = qT[c // 2][:, (c % 2) * 512:(c % 2 + 1) * 512]
            if c < 2:
                nc.scalar.activation(out=dest, in_=pt, func=AF.Relu,
                                     bias=biases["q"], scale=1.0)
            else:
                nc.vector.tensor_scalar(dest, pt, biases["q"], 0.0,
                                        op0=ALU.add, op1=ALU.max)

    # prologue projections: just enough for the first two stripes
    proj("k", 0)
    proj("q", 0)
    proj("q", 1)
    proj("v", 0)

    e_tiles = {}
    p_tiles = {}

    def emit_s(t):
        """Score matmuls + exp for stripe t; exp pairs write one shared tile."""
        c, j = t // NJ, t % NJ
        sp = ps.tile([P, ICH], f32, tag="s", bufs=2, name=f"s{t}")
        for cc in range(2):
            nc.tensor.matmul(sp[:, cc * 512:(cc + 1) * 512],
                             lhsT=kT[:, j * P:(j + 1) * P],
                             rhs=qT[c][:, cc * 512:(cc + 1) * 512],
                             start=True, stop=True)
        if t % 2 == 0:
            e_tiles[t // 2] = e_pool.tile([P, 2, ICH], bf, tag="e",
                                          name=f"e{t // 2}")
        nc.scalar.activation(out=e_tiles[t // 2][:, t % 2], in_=sp,
                             func=AF.Exp)
        if t >= NT - 2:
            # tail stripes: per-stripe multiply so the last dependency
            # chain (exp -> mult -> matmul -> store) is as short as possible
            if t % 2 == 0:
                p_tiles[t // 2] = p_pool.tile([P, 2, ICH], bf, tag="p",
                                              name=f"p{t // 2}")
            nc.vector.tensor_tensor(
                out=p_tiles[t // 2][:, t % 2],
                in0=e_tiles[t // 2][:, t % 2],
                in1=mask_all[:, j, c * ICH:(c + 1) * ICH],
                op=ALU.mult)
        elif t % 2 == 1:
            # one FD=2048 multiply for the stripe pair; mask side is a
            # 3D AP over two adjacent j-rows of mask_all
            p_pair = p_pool.tile([P, 2, ICH], bf, tag="p", name=f"p{t // 2}")
            nc.vector.tensor_tensor(
                out=p_pair,
                in0=e_tiles[t // 2],
                in1=mask_all[:, j - 1:j + 1, c * ICH:(c + 1) * ICH],
                op=ALU.mult)
            p_tiles[t // 2] = p_pair

    emit_s(0)
    emit_s(1)

    # late projections, folded into the first loop bodies
    deferred = [("v", 1), ("k", 1), ("v", 2), ("k", 2),
                ("v", 3), ("k", 3), ("q", 2), ("q", 3)]

    state = {"o": None, "r": None, "p01": None}

    def emit_r(rhs, start, stop):
        for cc in range(2):
            nc.tensor.matmul(state["r"][:, cc * 512:(cc + 1) * 512],
                             lhsT=ones_col,
                             rhs=rhs[:, cc * 512:(cc + 1) * 512],
                             start=start, stop=stop)

    def emit_o(t):
        """Out-matmuls (and rowsum reduction tree) for stripe t."""
        c, j = t // NJ, t % NJ
        i0 = c * ICH
        if j == 0:
            state["o"] = ps.tile([P, ICH], f32, tag="o", bufs=1, name=f"o{c}")
        o_ps = state["o"]
        p_t = p_tiles[t // 2][:, t % 2]

        for cc in range(2):
            nc.tensor.matmul(o_ps[:, cc * 512:(cc + 1) * 512],
                             lhsT=vN(j), rhs=p_t[:, cc * 512:(cc + 1) * 512],
                             start=(j == 0), stop=(j == NJ - 1))

        if t == NT - 3:
            # second-to-last pair feeds the rowsum directly (no quad)
            pr = p_tiles[t // 2]
            p01 = pp_pool.tile([P, ICH], bf, tag="q4", name=f"p01_{t}")
            nc.vector.tensor_tensor(out=p01, in0=pr[:, 0], in1=pr[:, 1],
                                    op=ALU.add)
            emit_r(p01, start=False, stop=False)
        elif t >= NT - 2:
            # last two stripes go straight into the rowsum accumulator,
            # keeping the final dependency chain minimal
            emit_r(p_t, start=False, stop=(t == NT - 1))

        if j % 4 == 3 and t < NT - 4:
            # rowsum reduction tree: one FD=2048 add over the two stripe
            # pairs, one FD=1024 add folding its halves, then the matmul
            pr0, pr1 = p_tiles[t // 2 - 1], p_tiles[t // 2]
            s2 = pp_pool.tile([P, 2, ICH], bf, tag="s2", name=f"s2_{t}")
            nc.vector.tensor_tensor(out=s2, in0=pr0, in1=pr1, op=ALU.add)
            del p_tiles[t // 2 - 1], p_tiles[t // 2]
            q4 = pp_pool.tile([P, ICH], bf, tag="q4", name=f"q4_{t}")
            nc.vector.tensor_tensor(out=q4, in0=s2[:, 0], in1=s2[:, 1],
                                    op=ALU.add)
            if j == 3:
                state["r"] = ps.tile([1, ICH], f32, tag="r", bufs=1,
                                     name=f"r{c}")
            emit_r(q4, start=(j == 3), stop=(j == NJ - 1 and t < NT - 1))

        if j == NJ - 1:
            # drain this chunk: rowsum first (ready before the last o-mm),
            # then outT quarters. Mid-kernel (chunk 0) everything stays on
            # DVE + the GPSIMD ring so the ACT exp stream is undisturbed;
            # the final chunk splits across DVE+ACT and both DMA rings.
            last = t == NT - 1
            rs_sb = drain_pool.tile([1, ICH], f32, tag="rs", name=f"rs{c}")
            if last:
                nc.vector.tensor_copy(out=rs_sb[:, 0:512],
                                      in_=state["r"][:, 0:512])
                nc.scalar.activation(out=rs_sb[:, 512:1024],
                                     in_=state["r"][:, 512:1024],
                                     func=AF.Copy)
            else:
                nc.vector.tensor_copy(out=rs_sb, in_=state["r"])
            nc.gpsimd.dma_start(out=rowsum[:, i0:i0 + ICH], in_=rs_sb)
            for cc in range(4):
                osb = drain_pool.tile([P, 256], f32, tag="osb", bufs=4,
                                      name=f"osb{c}_{cc}")
                piece = o_ps[:, cc * 256:(cc + 1) * 256]
                if last and cc % 2 == 1:
                    nc.scalar.activation(out=osb, in_=piece, func=AF.Copy)
                else:
                    nc.vector.tensor_copy(out=osb, in_=piece)
                if last:
                    # spread the final stores across all three DMA rings
                    eng = (nc.sync, nc.scalar, nc.gpsimd, nc.scalar)[cc]
                else:
                    eng = nc.gpsimd
                eng.dma_start(
                    out=outT[:, i0 + cc * 256:i0 + (cc + 1) * 256], in_=osb)

    for t in range(NT):
        # scores first so the exp-feeding path never queues behind the
        # projection bursts; the projections fill the out-matmul wait slots
        if t + 2 < NT:
            emit_s(t + 2)
        if t < 4:
            proj(*deferred[2 * t])
            proj(*deferred[2 * t + 1])
        if t >= 1:
            emit_o(t - 1)
    emit_o(NT - 1)


def _build_nc():
    if "nc" in _NC_CACHE:
        return _NC_CACHE["nc"]
    nc = bacc.Bacc("TRN2", target_bir_lowering=False, debug=False, num_devices=B)
    xT = nc.dram_tensor("xT", [DIN, N], bf, kind="ExternalInput").ap()
    maskT = nc.dram_tensor("maskT", [N, N], bf, kind="ExternalInput").ap()
    wpack = nc.dram_tensor("wpack", [P, 896], bf, kind="ExternalInput").ap()
    bpack = nc.dram_tensor("bpack", [P, 2], f32, kind="ExternalInput").ap()
    outT = nc.dram_tensor("outT", [HID, N], f32, kind="ExternalOutput").ap()
    rowsum = nc.dram_tensor("rowsum", [1, N], f32, kind="ExternalOutput").ap()

    with tile.TileContext(nc) as tc:
        with ExitStack() as ctx:
            _attention_tile_kernel(ctx, tc, outT, rowsum, xT, maskT,
                                   wpack, bpack)
    nc.compile()
    _NC_CACHE["nc"] = nc
    return nc


def make_in_maps(x, mask, Wv, bv, Wk, bk, Wq, bq):
    x = np.asarray(x, dtype=np.float32)
    mask = np.asarray(mask, dtype=np.float32)
    Wv = np.asarray(Wv, dtype=np.float32)
    bv = np.asarray(bv, dtype=np.float32)
    Wk = np.asarray(Wk, dtype=np.float32)
    bk = np.asarray(bk, dtype=np.float32)
    Wq = np.asarray(Wq, dtype=np.float32)
    bq = np.asarray(bq, dtype=np.float32)

    wpack = np.zeros((P, 896), dtype=ml_dtypes.bfloat16)
    wpack[:, 0:128] = Wq.T.astype(ml_dtypes.bfloat16)
    wpack[:, 128:256] = Wk.T.astype(ml_dtypes.bfloat16)
    wpack[:, 256:384] = Wv.T.astype(ml_dtypes.bfloat16)
    wpack[0, 384:896] = np.tile(bv, 4).astype(ml_dtypes.bfloat16)
    bpack = np.stack([bq, bk], axis=1).astype(np.float32)
    bpack = np.ascontiguousarray(bpack)

    in_maps = []
    for c in range(B):
        in_maps.append({
            "xT": np.ascontiguousarray(x[c].T.astype(ml_dtypes.bfloat16)),
            "maskT": np.ascontiguousarray(mask[c].T.astype(ml_dtypes.bfloat16)),
            "wpack": wpack, "bpack": bpack,
        })
    return in_maps


def postprocess(res):
    out = np.empty((B, N, HID), dtype=np.float32)
    for c in range(B):
        outT = res.results[c]["outT"]
        rowsum = res.results[c]["rowsum"]
        rowsum = np.where(rowsum == 0.0, 1.0, rowsum)
        out[c] = (outT / rowsum).T
    return out


def kernel(x, mask, Wv, bv, Wk, bk, Wq, bq):
    nc = _build_nc()
    in_maps = make_in_maps(x, mask, Wv, bv, Wk, bk, Wq, bq)
    res = bass_utils.run_bass_kernel_spmd(nc, in_maps, core_ids=list(range(B)),
                                          trace=False)
    return postprocess(res)


# revision 45
# speedup vs baseline: 1.2130x; 1.2130x over previous
"""Trainium2 Bass kernel for nn_AttModel (masked attention GNN message passing).

Contract: kernel(**inputs) takes the FULL unsharded inputs (x [8,2048,128],
mask [8,2048,2048], Wv/Wk/Wq [128,128], bv/bk/bq [128]) and returns the full
output [8, 2048, 128] float32.

Strategy: data-parallel over batch B=8 across the 8 NeuronCores; the small
weight matrices are replicated. The device kernel runs a fully transposed
dataflow (scores computed as S^T per j-stripe) so no [N,N] transpose is ever
done on device; the host pre-transposes x, mask and the weights (pure layout
marshaling) and post-normalizes/transposes the returned outT/rowsum.

Per core (batch element b), bf16 data path, f32 PSUM accumulation:
  qT/kT = relu(W x^T + b) as [h, n] bf16
  v     = relu(x W^T + b) computed directly in natural [j, h] orientation
          (bias via a K=1 ones-row matmul into PSUM; relu on DVE)
  for i-chunk (1024) and j-stripe (128):
    sT = kT_j^T @ qT_chunk      (PE, bf16, PSUM f32)
    eT = exp(sT)                (ACT, PSUM -> SBUF bf16, two stripes/tile)
    pT = eT * maskT             (DVE, one FD=2048 multiply per stripe pair)
    outT_chunk   += v_j^T @ pT  (PE, accumulated in PSUM over stripes)
    p01 = pT_even + pT_odd (DVE); rowsum += 1^T @ p01 (PE, per pair)
  Host: out_b = (outT / rowsum)^T

DMA-dispatch economy (the sync sequencer pays ~0.7us per dma_start):
inputs are packed into 3 loads (wpack, bpack, xT) + 16 full-row mask tiles
reused by both i-chunks; output stores ride the idle GPSIMD (SWDGE) ring.
"""

from contextlib import ExitStack

import numpy as np
import ml_dtypes

import concourse.bass as bass
import concourse.bacc as bacc
import concourse.tile as tile
from concourse import mybir
from concourse import bass_utils

B = 8
P = 128
N = 2048
HID = 128
DIN = 128
NJ = N // P      # 16 j-stripes
ICH = 1024       # i-chunk width
NCH = N // ICH   # 2 i-chunks
NT = NCH * NJ    # 32 global stripes

f32 = mybir.dt.float32
bf = mybir.dt.bfloat16
AF = mybir.ActivationFunctionType
ALU = mybir.AluOpType

_NC_CACHE = {}


def _attention_tile_kernel(ctx, tc, outT, rowsum, xT, maskT, wpack, bpack):
    nc = tc.nc

    consts = ctx.enter_context(tc.tile_pool(name="consts", bufs=1))
    big = ctx.enter_context(tc.tile_pool(name="big", bufs=1))
    ps = ctx.enter_context(tc.tile_pool(name="ps", bufs=1, space="PSUM"))
    e_pool = ctx.enter_context(tc.tile_pool(name="ep", bufs=4))
    p_pool = ctx.enter_context(tc.tile_pool(name="pp", bufs=5))
    pp_pool = ctx.enter_context(tc.tile_pool(name="pairp", bufs=3))
    drain_pool = ctx.enter_context(tc.tile_pool(name="drainp", bufs=2))

    # pre-warm the exp table set while DMAs are in flight
    warm_in = consts.tile([P, 1], f32)
    nc.vector.memset(warm_in, 0.0)
    warm_out = consts.tile([P, 1], bf)
    nc.scalar.activation(out=warm_out, in_=warm_in, func=AF.Exp)

    ones_col = consts.tile([P, 1], bf)
    nc.vector.memset(ones_col, 1.0)
    ones_row = consts.tile([1, P], bf)
    nc.vector.memset(ones_row, 1.0)

    # packed inputs, dispatch-ordered by criticality on the sync ring
    wp = consts.tile([P, 896], bf)
    nc.sync.dma_start(out=wp, in_=wpack)
    wts = {"q": wp[:, 0:P], "k": wp[:, P:2 * P], "v": wp[:, 2 * P:3 * P]}
    bvR = wp[0:1, 384:896]

    xT_sb = big.tile([P, N], bf)
    nc.sync.dma_start(out=xT_sb[:, 0:ICH], in_=xT[:, 0:ICH])
    xc = [xT_sb[:, c * 512:(c + 1) * 512] for c in range(4)]

    bp = consts.tile([P, 2], f32)
    nc.gpsimd.dma_start(out=bp, in_=bpack)
    biases = {"q": bp[:, 0:1], "k": bp[:, 1:2]}

    nc.sync.dma_start(out=xT_sb[:, ICH:N], in_=xT[:, ICH:N])

    # all mask rows prefetched once, reused by both i-chunks
    mask_all = big.tile([P, NJ, N], bf)
    for j in range(NJ):
        nc.sync.dma_start(out=mask_all[:, j], in_=maskT[j * P:(j + 1) * P, :])

    qT = [big.tile([P, ICH], bf, name=f"qT{c}") for c in range(NCH)]
    kT = big.tile([P, N], bf)
    vb = [big.tile([P, 512], bf, name=f"vb{c}") for c in range(4)]

    def vN(j):
        return vb[j // 4][:, (j % 4) * P:(j % 4 + 1) * P]

    def proj(nm, c):
        """Project chunk c (columns 512c:512c+512) of q/k/v; relu into SBUF.

        Projections borrow the rowsum PSUM slot (tag "r"), which is unused
        until chunk0's first (deliberately deferred) rowsum matmul — this
        keeps the score ring free to pace purely on exp drains."""
        pt = ps.tile([P, 512], f32, tag="r", bufs=1, name=f"proj_{nm}{c}")
        if nm == "v":
            # bias broadcast (K=1 ones matmul) then x @ WvT per stripe on top
            nc.tensor.matmul(pt, lhsT=ones_row, rhs=bvR,
                             start=True, stop=False, skip_group_check=True)
            for jj in range(4):
                nc.tensor.matmul(pt[:, jj * P:(jj + 1) * P],
                                 lhsT=xc[c][:, jj * P:(jj + 1) * P],
                                 rhs=wts["v"], start=False, stop=True,
                                 skip_group_check=True)
            nc.vector.tensor_scalar(vb[c], pt, 0.0, None, op0=ALU.max)
            return
        nc.tensor.matmul(pt, lhsT=wts[nm], rhs=xc[c], start=True, stop=True)
        if nm == "k":
            if c == 0:
                nc.scalar.activation(out=kT[:, 0:512], in_=pt,
                                     func=AF.Relu, bias=biases["k"], scale=1.0)
            else:
                nc.vector.tensor_scalar(kT[:, c * 512:(c + 1) * 512], pt,
                                        biases["k"], 0.0,
                                        op0=ALU.add, op1=ALU.max)
        else:
            dest = qT[c // 2][:, (c % 2) * 512:(c % 2 + 1) * 512]
            if c < 2:
                nc.scalar.activation(out=dest, in_=pt, func=AF.Relu,
                                     bias=biases["q"], scale=1.0)
            else:
                nc.vector.tensor_scalar(dest, pt, biases["q"], 0.0,
                                        op0=ALU.add, op1=ALU.max)

    # prologue projections: just enough for the first two stripes
    proj("k", 0)
    proj("q", 0)
    proj("q", 1)
    proj("v", 0)

    e_tiles = {}
    p_tiles = {}

    def emit_s(t):
        """Score matmuls + exp for stripe t; exp pairs write one shared tile."""
        c, j = t // NJ, t % NJ
        sp = ps.tile([P, ICH], f32, tag="s", bufs=2, name=f"s{t}")
        for cc in range(2):
            nc.tensor.matmul(sp[:, cc * 512:(cc + 1) * 512],
                             lhsT=kT[:, j * P:(j + 1) * P],
                             rhs=qT[c][:, cc * 512:(cc + 1) * 512],
                             start=True, stop=True)
        if t % 2 == 0:
            e_tiles[t // 2] = e_pool.tile([P, 2, ICH], bf, tag="e",
                                          name=f"e{t // 2}")
        nc.scalar.activation(out=e_tiles[t // 2][:, t % 2], in_=sp,
                             func=AF.Exp)
        if t >= NT - 2:
            # tail stripes: per-stripe multiply so the last dependency
            # chain (exp -> mult -> matmul -> store) is as short as possible
            if t % 2 == 0:
                p_tiles[t // 2] = p_pool.tile([P, 2, ICH], bf, tag="p",
                                              name=f"p{t // 2}")
            nc.vector.tensor_tensor(
                out=p_tiles[t // 2][:, t % 2],
                in0=e_tiles[t // 2][:, t % 2],
                in1=mask_all[:, j, c * ICH:(c + 1) * ICH],
                op=ALU.mult)
        elif t % 2 == 1:
            # one FD=2048 multiply for the stripe pair; mask side is a
            # 3D AP over two adjacent j-rows of mask_all
            p_pair = p_pool.tile([P, 2, ICH], bf, tag="p", name=f"p{t // 2}")
            nc.vector.tensor_tensor(
                out=p_pair,
                in0=e_tiles[t // 2],
                in1=mask_all[:, j - 1:j + 1, c * ICH:(c + 1) * ICH],
                op=ALU.mult)
            p_tiles[t // 2] = p_pair

    emit_s(0)
    emit_s(1)

    # late projections, folded into the first loop bodies (k1 first: it has
    # the earliest deadline, the j=4 score matmul)
    deferred = [("k", 1), ("v", 1), ("k", 2), ("v", 2),
                ("k", 3), ("v", 3), ("q", 2), ("q", 3)]

    state = {"o": None, "r": None, "p01": None}
    r_stash = []

    def emit_r(rhs, start, stop):
        for cc in range(2):
            nc.tensor.matmul(state["r"][:, cc * 512:(cc + 1) * 512],
                             lhsT=ones_col,
                             rhs=rhs[:, cc * 512:(cc + 1) * 512],
                             start=start, stop=stop)

    def emit_o(t):
        """Out-matmuls (and rowsum reduction tree) for stripe t."""
        c, j = t // NJ, t % NJ
        i0 = c * ICH
        if j == 0:
            state["o"] = ps.tile([P, ICH], f32, tag="o", bufs=1, name=f"o{c}")
        o_ps = state["o"]
        p_t = p_tiles[t // 2][:, t % 2]

        for cc in range(2):
            nc.tensor.matmul(o_ps[:, cc * 512:(cc + 1) * 512],
                             lhsT=vN(j), rhs=p_t[:, cc * 512:(cc + 1) * 512],
                             start=(j == 0), stop=(j == NJ - 1))

        if t == NT - 3:
            # second-to-last pair feeds the rowsum directly (no quad)
            pr = p_tiles[t // 2]
            p01 = pp_pool.tile([P, ICH], bf, tag="q4", name=f"p01_{t}")
            nc.vector.tensor_tensor(out=p01, in0=pr[:, 0], in1=pr[:, 1],
                                    op=ALU.add)
            emit_r(p01, start=False, stop=False)
        elif t >= NT - 2:
            # last two stripes go straight into the rowsum accumulator,
            # keeping the final dependency chain minimal
            emit_r(p_t, start=False, stop=(t == NT - 1))

        if j % 4 == 3 and t < NT - 4:
            # rowsum reduction tree: one FD=2048 add over the two stripe
            # pairs, one FD=1024 add folding its halves, then the matmul
            pr0, pr1 = p_tiles[t // 2 - 1], p_tiles[t // 2]
            s2 = pp_pool.tile([P, 2, ICH], bf, tag="s2", name=f"s2_{t}")
            nc.vector.tensor_tensor(out=s2, in0=pr0, in1=pr1, op=ALU.add)
            del p_tiles[t // 2 - 1], p_tiles[t // 2]
            q4 = pp_pool.tile([P, ICH], bf, tag="q4", name=f"q4_{t}")
            nc.vector.tensor_tensor(out=q4, in0=s2[:, 0], in1=s2[:, 1],
                                    op=ALU.add)
            if c == 0 and j < NJ - 1:
                # chunk0's rowsum matmuls are deferred to later bodies so
                # the "r" PSUM slot stays free for the projections; the q4
                # tiles persist in SBUF until then
                r_stash.append(q4)
            else:
                if j == 3:
                    state["r"] = ps.tile([1, ICH], f32, tag="r", bufs=1,
                                         name=f"r{c}")
                emit_r(q4, start=(j == 3 and c > 0),
                       stop=(j == NJ - 1 and t < NT - 1))

        if j == NJ - 1:
            # drain this chunk: outT pieces first (they only need the last
            # o-matmul, and releasing the o accumulator early unblocks the
            # next chunk), then the rowsum after its final matmul
            last = t == NT - 1
            for cc in range(4):
                osb = drain_pool.tile([P, 256], f32, tag="osb", bufs=4,
                                      name=f"osb{c}_{cc}")
                piece = o_ps[:, cc * 256:(cc + 1) * 256]
                if last and cc % 2 == 1:
                    nc.scalar.activation(out=osb, in_=piece, func=AF.Copy)
                else:
                    nc.vector.tensor_copy(out=osb, in_=piece)
                if last:
                    # spread the final stores across all three DMA rings
                    eng = (nc.sync, nc.scalar, nc.gpsimd, nc.scalar)[cc]
                else:
                    eng = nc.gpsimd
                eng.dma_start(
                    out=outT[:, i0 + cc * 256:i0 + (cc + 1) * 256], in_=osb)
            rs_sb = drain_pool.tile([1, ICH], f32, tag="rs", name=f"rs{c}")
            if last:
                nc.vector.tensor_copy(out=rs_sb[:, 0:512],
                                      in_=state["r"][:, 0:512])
                nc.scalar.activation(out=rs_sb[:, 512:1024],
                                     in_=state["r"][:, 512:1024],
                                     func=AF.Copy)
            else:
                nc.vector.tensor_copy(out=rs_sb, in_=state["r"])
            nc.gpsimd.dma_start(out=rowsum[:, i0:i0 + ICH], in_=rs_sb)

    for t in range(NT):
        # scores first so the exp-feeding path never queues behind the
        # projection bursts; the projections fill the out-matmul wait slots
        if t + 2 < NT:
            emit_s(t + 2)
        if t < 4:
            proj(*deferred[2 * t])
            proj(*deferred[2 * t + 1])
        if t in (11, 13, 15) and r_stash:
            # deferred chunk0 rowsum matmuls (the "r" slot is free now)
            q4s = r_stash.pop(0)
            first = state["r"] is None
            if first:
                state["r"] = ps.tile([1, ICH], f32, tag="r", bufs=1,
                                     name="r0")
            emit_r(q4s, start=first, stop=False)
        if t >= 1:
            emit_o(t - 1)
    emit_o(NT - 1)


def _build_nc():
    if "nc" in _NC_CACHE:
        return _NC_CACHE["nc"]
    nc = bacc.Bacc("TRN2", target_bir_lowering=False, debug=False, num_devices=B)
    xT = nc.dram_tensor("xT", [DIN, N], bf, kind="ExternalInput").ap()
    maskT = nc.dram_tensor("maskT", [N, N], bf, kind="ExternalInput").ap()
    wpack = nc.dram_tensor("wpack", [P, 896], bf, kind="ExternalInput").ap()
    bpack = nc.dram_tensor("bpack", [P, 2], f32, kind="ExternalInput").ap()
    outT = nc.dram_tensor("outT", [HID, N], f32, kind="ExternalOutput").ap()
    rowsum = nc.dram_tensor("rowsum", [1, N], f32, kind="ExternalOutput").ap()

    with tile.TileContext(nc) as tc:
        with ExitStack() as ctx:
            _attention_tile_kernel(ctx, tc, outT, rowsum, xT, maskT,
                                   wpack, bpack)
    nc.compile()
    _NC_CACHE["nc"] = nc
    return nc


def make_in_maps(x, mask, Wv, bv, Wk, bk, Wq, bq):
    x = np.asarray(x, dtype=np.float32)
    mask = np.asarray(mask, dtype=np.float32)
    Wv = np.asarray(Wv, dtype=np.float32)
    bv = np.asarray(bv, dtype=np.float32)
    Wk = np.asarray(Wk, dtype=np.float32)
    bk = np.asarray(bk, dtype=np.float32)
    Wq = np.asarray(Wq, dtype=np.float32)
    bq = np.asarray(bq, dtype=np.float32)

    wpack = np.zeros((P, 896), dtype=ml_dtypes.bfloat16)
    wpack[:, 0:128] = Wq.T.astype(ml_dtypes.bfloat16)
    wpack[:, 128:256] = Wk.T.astype(ml_dtypes.bfloat16)
    wpack[:, 256:384] = Wv.T.astype(ml_dtypes.bfloat16)
    wpack[0, 384:896] = np.tile(bv, 4).astype(ml_dtypes.bfloat16)
    bpack = np.stack([bq, bk], axis=1).astype(np.float32)
    bpack = np.ascontiguousarray(bpack)

    in_maps = []
    for c in range(B):
        in_maps.append({
            "xT": np.ascontiguousarray(x[c].T.astype(ml_dtypes.bfloat16)),
            "maskT": np.ascontiguousarray(mask[c].T.astype(ml_dtypes.bfloat16)),
            "wpack": wpack, "bpack": bpack,
        })
    return in_maps


def postprocess(res):
    out = np.empty((B, N, HID), dtype=np.float32)
    for c in range(B):
        outT = res.results[c]["outT"]
        rowsum = res.results[c]["rowsum"]
        rowsum = np.where(rowsum == 0.0, 1.0, rowsum)
        out[c] = (outT / rowsum).T
    return out


def kernel(x, mask, Wv, bv, Wk, bk, Wq, bq):
    nc = _build_nc()
    in_maps = make_in_maps(x, mask, Wv, bv, Wk, bk, Wq, bq)
    res = bass_utils.run_bass_kernel_spmd(nc, in_maps, core_ids=list(range(B)),
                                          trace=False)
    return postprocess(res)


# revision 48
# speedup vs baseline: 1.2954x; 1.0679x over previous
"""Trainium2 Bass kernel for nn_AttModel (masked attention GNN message passing).

Contract: kernel(**inputs) takes the FULL unsharded inputs (x [8,2048,128],
mask [8,2048,2048], Wv/Wk/Wq [128,128], bv/bk/bq [128]) and returns the full
output [8, 2048, 128] float32.

Strategy: data-parallel over batch B=8 across the 8 NeuronCores; the small
weight matrices are replicated. The device kernel runs a fully transposed
dataflow (scores computed as S^T per j-stripe) so no [N,N] transpose is ever
done on device; the host pre-transposes x, mask and the weights (pure layout
marshaling) and post-normalizes/transposes the returned outT/rowsum.

Per core (batch element b), bf16 data path, f32 PSUM accumulation:
  qT/kT = relu(W x^T + b) as [h, n] bf16
  v     = relu(x W^T + b) computed directly in natural [j, h] orientation
          (bias via a K=1 ones-row matmul into PSUM; relu on DVE)
  for i-chunk (1024) and j-stripe (128):
    sT = kT_j^T @ qT_chunk      (PE, bf16, PSUM f32)
    eT = exp(sT)                (ACT, PSUM -> SBUF bf16, two stripes/tile)
    pT = eT * maskT             (DVE, one FD=2048 multiply per stripe pair)
    outT_chunk   += v_j^T @ pT  (PE, accumulated in PSUM over stripes)
    p01 = pT_even + pT_odd (DVE); rowsum += 1^T @ p01 (PE, per pair)
  Host: out_b = (outT / rowsum)^T

DMA-dispatch economy (the sync sequencer pays ~0.7us per dma_start):
inputs are packed into 3 loads (wpack, bpack, xT) + 16 full-row mask tiles
reused by both i-chunks; output stores ride the idle GPSIMD (SWDGE) ring.
"""

from contextlib import ExitStack

import numpy as np
import ml_dtypes

import concourse.bass as bass
import concourse.bacc as bacc
import concourse.tile as tile
from concourse import mybir
from concourse import bass_utils

B = 8
P = 128
N = 2048
HID = 128
DIN = 128
NJ = N // P      # 16 j-stripes
ICH = 1024       # i-chunk width
NCH = N // ICH   # 2 i-chunks
NT = NCH * NJ    # 32 global stripes

f32 = mybir.dt.float32
bf = mybir.dt.bfloat16
AF = mybir.ActivationFunctionType
ALU = mybir.AluOpType

_NC_CACHE = {}


def _attention_tile_kernel(ctx, tc, outT, rowsum, xT, maskT, wpack, bpack):
    nc = tc.nc

    consts = ctx.enter_context(tc.tile_pool(name="consts", bufs=1))
    big = ctx.enter_context(tc.tile_pool(name="big", bufs=1))
    ps = ctx.enter_context(tc.tile_pool(name="ps", bufs=1, space="PSUM"))
    e_pool = ctx.enter_context(tc.tile_pool(name="ep", bufs=4))
    p_pool = ctx.enter_context(tc.tile_pool(name="pp", bufs=5))
    pp_pool = ctx.enter_context(tc.tile_pool(name="pairp", bufs=3))
    drain_pool = ctx.enter_context(tc.tile_pool(name="drainp", bufs=2))

    # pre-warm the exp table set while DMAs are in flight
    warm_in = consts.tile([P, 1], f32)
    nc.vector.memset(warm_in, 0.0)
    warm_out = consts.tile([P, 1], bf)
    nc.scalar.activation(out=warm_out, in_=warm_in, func=AF.Exp)

    ones_col = consts.tile([P, 1], bf)
    nc.vector.memset(ones_col, 1.0)
    ones_row = consts.tile([1, P], bf)
    nc.vector.memset(ones_row, 1.0)

    # packed inputs, dispatch-ordered by criticality on the sync ring
    wp = consts.tile([P, 896], bf)
    nc.sync.dma_start(out=wp, in_=wpack)
    wts = {"q": wp[:, 0:P], "k": wp[:, P:2 * P], "v": wp[:, 2 * P:3 * P]}
    bvR = wp[0:1, 384:896]

    xT_sb = big.tile([P, N], bf)
    nc.sync.dma_start(out=xT_sb[:, 0:ICH], in_=xT[:, 0:ICH])
    xc = [xT_sb[:, c * 512:(c + 1) * 512] for c in range(4)]

    bp = consts.tile([P, 2], f32)
    nc.gpsimd.dma_start(out=bp, in_=bpack)
    biases = {"q": bp[:, 0:1], "k": bp[:, 1:2]}

    nc.sync.dma_start(out=xT_sb[:, ICH:N], in_=xT[:, ICH:N])

    # all mask rows prefetched once, reused by both i-chunks
    mask_all = big.tile([P, NJ, N], bf)
    for j in range(NJ):
        nc.sync.dma_start(out=mask_all[:, j], in_=maskT[j * P:(j + 1) * P, :])

    qT = [big.tile([P, ICH], bf, name=f"qT{c}") for c in range(NCH)]
    kT = big.tile([P, N], bf)
    vb = [big.tile([P, 512], bf, name=f"vb{c}") for c in range(4)]

    def vN(j):
        return vb[j // 4][:, (j % 4) * P:(j % 4 + 1) * P]

    def proj(nm, c):
        """Project chunk c (columns 512c:512c+512) of q/k/v; relu into SBUF.

        Projections borrow the rowsum PSUM slot (tag "r"), which is unused
        until chunk0's first (deliberately deferred) rowsum matmul — this
        keeps the score ring free to pace purely on exp drains. The three
        prologue q/k projections use the score ring instead (it is empty,
        and one-per-body s allocations keep its parity clean afterwards)."""
        if nm != "v" and c == 0 or (nm == "q" and c == 1):
            pt = ps.tile([P, 512], f32, tag="s", bufs=2, name=f"proj_{nm}{c}")
        else:
            pt = ps.tile([P, 512], f32, tag="r", bufs=1, name=f"proj_{nm}{c}")
        if nm == "v":
            # bias broadcast (K=1 ones matmul) then x @ WvT per stripe on top
            nc.tensor.matmul(pt, lhsT=ones_row, rhs=bvR,
                             start=True, stop=False, skip_group_check=True)
            for jj in range(4):
                nc.tensor.matmul(pt[:, jj * P:(jj + 1) * P],
                                 lhsT=xc[c][:, jj * P:(jj + 1) * P],
                                 rhs=wts["v"], start=False, stop=True,
                                 skip_group_check=True)
            nc.vector.tensor_scalar(vb[c], pt, 0.0, None, op0=ALU.max)
            return
        nc.tensor.matmul(pt, lhsT=wts[nm], rhs=xc[c], start=True, stop=True)
        if nm == "k":
            if c == 0:
                nc.scalar.activation(out=kT[:, 0:512], in_=pt,
                                     func=AF.Relu, bias=biases["k"], scale=1.0)
            else:
                nc.vector.tensor_scalar(kT[:, c * 512:(c + 1) * 512], pt,
                                        biases["k"], 0.0,
                                        op0=ALU.add, op1=ALU.max)
        else:
            dest = qT[c // 2][:, (c % 2) * 512:(c % 2 + 1) * 512]
            if c < 2:
                nc.scalar.activation(out=dest, in_=pt, func=AF.Relu,
                                     bias=biases["q"], scale=1.0)
            else:
                nc.vector.tensor_scalar(dest, pt, biases["q"], 0.0,
                                        op0=ALU.add, op1=ALU.max)

    # prologue projections: just enough for the first two stripes (v0 comes
    # after the first scores — it lives in the independent "r" ring)
    proj("k", 0)
    proj("q", 0)
    proj("q", 1)

    e_tiles = {}
    p_tiles = {}

    def emit_s(t):
        """Score matmuls + exp for stripe t; exp pairs write one shared tile."""
        c, j = t // NJ, t % NJ
        sp = ps.tile([P, ICH], f32, tag="s", bufs=2, name=f"s{t}")
        for cc in range(2):
            nc.tensor.matmul(sp[:, cc * 512:(cc + 1) * 512],
                             lhsT=kT[:, j * P:(j + 1) * P],
                             rhs=qT[c][:, cc * 512:(cc + 1) * 512],
                             start=True, stop=True)
        if t % 2 == 0:
            e_tiles[t // 2] = e_pool.tile([P, 2, ICH], bf, tag="e",
                                          name=f"e{t // 2}")
        nc.scalar.activation(out=e_tiles[t // 2][:, t % 2], in_=sp,
                             func=AF.Exp)
        if t >= NT - 2:
            # tail stripes: per-stripe multiply so the last dependency
            # chain (exp -> mult -> matmul -> store) is as short as possible
            if t % 2 == 0:
                p_tiles[t // 2] = p_pool.tile([P, 2, ICH], bf, tag="p",
                                              name=f"p{t // 2}")
            nc.vector.tensor_tensor(
                out=p_tiles[t // 2][:, t % 2],
                in0=e_tiles[t // 2][:, t % 2],
                in1=mask_all[:, j, c * ICH:(c + 1) * ICH],
                op=ALU.mult)
        elif t % 2 == 1:
            # one FD=2048 multiply for the stripe pair; mask side is a
            # 3D AP over two adjacent j-rows of mask_all
            p_pair = p_pool.tile([P, 2, ICH], bf, tag="p", name=f"p{t // 2}")
            nc.vector.tensor_tensor(
                out=p_pair,
                in0=e_tiles[t // 2],
                in1=mask_all[:, j - 1:j + 1, c * ICH:(c + 1) * ICH],
                op=ALU.mult)
            p_tiles[t // 2] = p_pair

    emit_s(0)
    emit_s(1)
    proj("v", 0)

    # late projections, folded into the first loop bodies (k1 first: it has
    # the earliest deadline, the j=4 score matmul)
    deferred = [("k", 1), ("v", 1), ("k", 2), ("v", 2),
                ("k", 3), ("v", 3), ("q", 2), ("q", 3)]

    state = {"o": None, "r": None, "p01": None}
    r_stash = []

    def emit_r(rhs, start, stop):
        for cc in range(2):
            nc.tensor.matmul(state["r"][:, cc * 512:(cc + 1) * 512],
                             lhsT=ones_col,
                             rhs=rhs[:, cc * 512:(cc + 1) * 512],
                             start=start, stop=stop)

    def emit_o(t):
        """Out-matmuls (and rowsum reduction tree) for stripe t."""
        c, j = t // NJ, t % NJ
        i0 = c * ICH
        if j == 0:
            state["o"] = ps.tile([P, ICH], f32, tag="o", bufs=1, name=f"o{c}")
        o_ps = state["o"]
        p_t = p_tiles[t // 2][:, t % 2]

        for cc in range(2):
            nc.tensor.matmul(o_ps[:, cc * 512:(cc + 1) * 512],
                             lhsT=vN(j), rhs=p_t[:, cc * 512:(cc + 1) * 512],
                             start=(j == 0), stop=(j == NJ - 1))

        if t == NT - 3:
            # second-to-last pair feeds the rowsum directly (no quad)
            pr = p_tiles[t // 2]
            p01 = pp_pool.tile([P, ICH], bf, tag="q4", name=f"p01_{t}")
            nc.vector.tensor_tensor(out=p01, in0=pr[:, 0], in1=pr[:, 1],
                                    op=ALU.add)
            emit_r(p01, start=False, stop=False)
        elif t >= NT - 2:
            # last two stripes go straight into the rowsum accumulator,
            # keeping the final dependency chain minimal
            emit_r(p_t, start=False, stop=(t == NT - 1))

        if j % 4 == 3 and t < NT - 4:
            # rowsum reduction tree: one FD=2048 add over the two stripe
            # pairs, one FD=1024 add folding its halves, then the matmul
            pr0, pr1 = p_tiles[t // 2 - 1], p_tiles[t // 2]
            s2 = pp_pool.tile([P, 2, ICH], bf, tag="s2", name=f"s2_{t}")
            nc.vector.tensor_tensor(out=s2, in0=pr0, in1=pr1, op=ALU.add)
            del p_tiles[t // 2 - 1], p_tiles[t // 2]
            q4 = pp_pool.tile([P, ICH], bf, tag="q4", name=f"q4_{t}")
            nc.vector.tensor_tensor(out=q4, in0=s2[:, 0], in1=s2[:, 1],
                                    op=ALU.add)
            if c == 0 and j < NJ - 1:
                # chunk0's rowsum matmuls are deferred to later bodies so
                # the "r" PSUM slot stays free for the projections; the q4
                # tiles persist in SBUF until then
                r_stash.append(q4)
            else:
                if j == 3:
                    state["r"] = ps.tile([1, ICH], f32, tag="r", bufs=1,
                                         name=f"r{c}")
                emit_r(q4, start=(j == 3 and c > 0),
                       stop=(j == NJ - 1 and t < NT - 1))

        if j == NJ - 1:
            # drain this chunk: outT pieces first (they only need the last
            # o-matmul, and releasing the o accumulator early unblocks the
            # next chunk), then the rowsum after its final matmul
            last = t == NT - 1
            for cc in range(4):
                osb = drain_pool.tile([P, 256], f32, tag="osb", bufs=4,
                                      name=f"osb{c}_{cc}")
                piece = o_ps[:, cc * 256:(cc + 1) * 256]
                if last and cc % 2 == 1:
                    nc.scalar.activation(out=osb, in_=piece, func=AF.Copy)
                else:
                    nc.vector.tensor_copy(out=osb, in_=piece)
                if last:
                    # spread the final stores across all three DMA rings
                    eng = (nc.sync, nc.scalar, nc.gpsimd, nc.scalar)[cc]
                else:
                    eng = nc.gpsimd
                eng.dma_start(
                    out=outT[:, i0 + cc * 256:i0 + (cc + 1) * 256], in_=osb)
            rs_sb = drain_pool.tile([1, ICH], f32, tag="rs", name=f"rs{c}")
            if last:
                nc.vector.tensor_copy(out=rs_sb[:, 0:512],
                                      in_=state["r"][:, 0:512])
                nc.scalar.activation(out=rs_sb[:, 512:1024],
                                     in_=state["r"][:, 512:1024],
                                     func=AF.Copy)
            else:
                nc.vector.tensor_copy(out=rs_sb, in_=state["r"])
            nc.gpsimd.dma_start(out=rowsum[:, i0:i0 + ICH], in_=rs_sb)

    for t in range(NT):
        # scores first so the exp-feeding path never queues behind the
        # projection bursts; the projections fill the out-matmul wait slots
        if t + 2 < NT:
            emit_s(t + 2)
        if t < 4:
            proj(*deferred[2 * t])
            proj(*deferred[2 * t + 1])
        if t in (11, 13, 15) and r_stash:
            # deferred chunk0 rowsum matmuls (the "r" slot is free now)
            q4s = r_stash.pop(0)
            first = state["r"] is None
            if first:
                state["r"] = ps.tile([1, ICH], f32, tag="r", bufs=1,
                                     name="r0")
            emit_r(q4s, start=first, stop=False)
        if t >= 1:
            emit_o(t - 1)
    emit_o(NT - 1)


def _build_nc():
    if "nc" in _NC_CACHE:
        return _NC_CACHE["nc"]
    nc = bacc.Bacc("TRN2", target_bir_lowering=False, debug=False, num_devices=B)
    xT = nc.dram_tensor("xT", [DIN, N], bf, kind="ExternalInput").ap()
    maskT = nc.dram_tensor("maskT", [N, N], bf, kind="ExternalInput").ap()
    wpack = nc.dram_tensor("wpack", [P, 896], bf, kind="ExternalInput").ap()
    bpack = nc.dram_tensor("bpack", [P, 2], f32, kind="ExternalInput").ap()
    outT = nc.dram_tensor("outT", [HID, N], f32, kind="ExternalOutput").ap()
    rowsum = nc.dram_tensor("rowsum", [1, N], f32, kind="ExternalOutput").ap()

    with tile.TileContext(nc) as tc:
        with ExitStack() as ctx:
            _attention_tile_kernel(ctx, tc, outT, rowsum, xT, maskT,
                                   wpack, bpack)
    nc.compile()
    _NC_CACHE["nc"] = nc
    return nc


def make_in_maps(x, mask, Wv, bv, Wk, bk, Wq, bq):
    x = np.asarray(x, dtype=np.float32)
    mask = np.asarray(mask, dtype=np.float32)
    Wv = np.asarray(Wv, dtype=np.float32)
    bv = np.asarray(bv, dtype=np.float32)
    Wk = np.asarray(Wk, dtype=np.float32)
    bk = np.asarray(bk, dtype=np.float32)
    Wq = np.asarray(Wq, dtype=np.float32)
    bq = np.asarray(bq, dtype=np.float32)

    wpack = np.zeros((P, 896), dtype=ml_dtypes.bfloat16)
    wpack[:, 0:128] = Wq.T.astype(ml_dtypes.bfloat16)
    wpack[:, 128:256] = Wk.T.astype(ml_dtypes.bfloat16)
    wpack[:, 256:384] = Wv.T.astype(ml_dtypes.bfloat16)
    wpack[0, 384:896] = np.tile(bv, 4).astype(ml_dtypes.bfloat16)
    bpack = np.stack([bq, bk], axis=1).astype(np.float32)
    bpack = np.ascontiguousarray(bpack)

    in_maps = []
    for c in range(B):
        in_maps.append({
            "xT": np.ascontiguousarray(x[c].T.astype(ml_dtypes.bfloat16)),
            "maskT": np.ascontiguousarray(mask[c].T.astype(ml_dtypes.bfloat16)),
            "wpack": wpack, "bpack": bpack,
        })
    return in_maps


def postprocess(res):
    out = np.empty((B, N, HID), dtype=np.float32)
    for c in range(B):
        outT = res.results[c]["outT"]
        rowsum = res.results[c]["rowsum"]
        rowsum = np.where(rowsum == 0.0, 1.0, rowsum)
        out[c] = (outT / rowsum).T
    return out


def kernel(x, mask, Wv, bv, Wk, bk, Wq, bq):
    nc = _build_nc()
    in_maps = make_in_maps(x, mask, Wv, bv, Wk, bk, Wq, bq)
    res = bass_utils.run_bass_kernel_spmd(nc, in_maps, core_ids=list(range(B)),
                                          trace=False)
    return postprocess(res)
